# revision 1
# baseline (speedup 1.0000x reference)
"""Masked multi-head attention on 8 Trainium2 NeuronCores.

Reference computation (fp32):
    qkv = x @ W_qkv + b_qkv ; split q,k,v ; 16 heads, dh=64
    attn = softmax(causal(q k^T / 8)) ; z = attn v ; out = z @ W_proj + b_proj

Sharding: tensor-parallel over heads. Core c owns heads {2c, 2c+1}
(columns 128c:128c+128 of each of the q/k/v blocks of W_qkv, rows
128c:128c+128 of W_proj). Each core computes its heads' attention and a
partial output projection; the host sums the 8 partials and adds b_proj.

On-device layout highlights:
  - x is fed pre-transposed (host) as xt[b] = x[b].T so the QKV matmuls
    contract over d with d on partitions: q^T = W_q^T @ x^T comes out in
    [e, t] layout directly (e = 2*64 head channels on partitions).
  - Scores are computed transposed, S^T[k, q] = (k^T)^T q^T per head, so
    softmax's denominator is produced by an ones-augmented AV matmul:
    lhsT = [v | 1] gives z'^T = [z^T ; sum_j P] in one accumulation.
  - exp (with the 1/8 scale folded in) runs on the scalar engine straight
    out of PSUM; causal masking multiplies precomputed 0/1 tiles on the
    diagonal blocks only; fully-masked tiles are never computed.
  - All matmuls run in fp32r (full PE rate at N>=512, ~2^-12 rounding).

The harness-visible entry point is kernel(**inputs) -> np.ndarray.
"""

import sys

sys.path.insert(0, "/opt/trn_rl_repo")

import math

import numpy as np

B = 2
S = 2048
D = 1024
NH = 16
DH = 64
NCORES = 8
TT = 512  # t-tile for qkv / q-tile for scores
NQI = S // TT  # 4
NKJ = S // 128  # 16


def _legalize_multi_waits(nc, max_waits=1):
    """This container's walrus rejects >1 semaphore wait per instruction
    (CoreV3 setupSyncWait "Too many sync wait commands"). Hoist extras
    onto same-engine NOPs inserted right before the offending one."""
    import concourse.mybir as mybir

    n_fixed = 0
    for fn in nc.m.functions:
        for blk in fn.blocks:
            new_insts = []
            for inst in blk.instructions:
                si = inst.sync_info
                waits = list(si.on_wait) if si is not None else []
                if len(waits) > max_waits:
                    extra, keep = waits[:-max_waits], waits[-max_waits:]
                    k = 0
                    while extra:
                        chunk, extra = extra[:max_waits], extra[max_waits:]
                        new_insts.append(
                            mybir.InstNoOp(
                                name=f"{inst.name}-wsplit{k}",
                                engine=inst.engine,
                                ins=[],
                                outs=[],
                                sync_info=mybir.SyncInfo(on_wait=chunk, on_update=[]),
                            )
                        )
                        k += 1
                    inst.sync_info = mybir.SyncInfo(
                        on_wait=keep, on_update=list(si.on_update)
                    )
                    n_fixed += 1
                new_insts.append(inst)
            blk.instructions = new_insts
    return n_fixed


def build_module(reps: int = 1):
    import concourse.bass as bass
    import concourse.mybir as mybir
    import concourse.tile as tile
    from concourse.bass import ts
    from concourse.masks import make_identity

    F32 = mybir.dt.float32
    F32R = mybir.dt.float32r
    Identity = mybir.ActivationFunctionType.Identity
    Exp = mybir.ActivationFunctionType.Exp

    nc = bass.Bass(
        trn_type="TRN2", target_bir_lowering=False, debug=False, num_devices=NCORES
    )

    xt = nc.dram_tensor("xt", [B, D, S], F32R, kind="ExternalInput").ap()
    wq = nc.dram_tensor("wq", [8, 128, 128], F32R, kind="ExternalInput").ap()
    wk = nc.dram_tensor("wk", [8, 128, 128], F32R, kind="ExternalInput").ap()
    wv = nc.dram_tensor("wv", [8, 128, 128], F32R, kind="ExternalInput").ap()
    bq = nc.dram_tensor("bq", [128, 1], F32, kind="ExternalInput").ap()
    bk = nc.dram_tensor("bk", [128, 1], F32, kind="ExternalInput").ap()
    bv = nc.dram_tensor("bv", [128, 1], F32, kind="ExternalInput").ap()
    wp = nc.dram_tensor("wp", [128, D], F32R, kind="ExternalInput").ap()
    msk = nc.dram_tensor("msk", [4, 128, 2 * TT], F32R, kind="ExternalInput").ap()
    ones = nc.dram_tensor("ones", [128, 64], F32R, kind="ExternalInput").ap()
    out = nc.dram_tensor("out", [B, S, D], F32, kind="ExternalOutput").ap()

    with tile.TileContext(nc) as tc:
        with (
            tc.tile_pool(name="const", bufs=1) as cpool,
            tc.tile_pool(name="work", bufs=1) as wpool,
            tc.tile_pool(name="p3", bufs=3) as p3,
            tc.tile_pool(name="p2", bufs=2) as p2,
            tc.tile_pool(name="ps2", bufs=2, space="PSUM") as ps2,
        ):
            # ---- constants (outside the timing loop) ----
            wq_t = cpool.tile([128, 8, 128], F32R, tag="wq")
            wk_t = cpool.tile([128, 8, 128], F32R, tag="wk")
            wv_t = cpool.tile([128, 8, 128], F32R, tag="wv")
            nc.sync.dma_start(wq_t[:], wq.rearrange("o p e -> p o e"))
            nc.sync.dma_start(wk_t[:], wk.rearrange("o p e -> p o e"))
            nc.sync.dma_start(wv_t[:], wv.rearrange("o p e -> p o e"))
            bq_t = cpool.tile([128, 1], F32, tag="bq")
            bk_t = cpool.tile([128, 1], F32, tag="bk")
            bv_t = cpool.tile([128, 1], F32, tag="bv")
            nc.sync.dma_start(bq_t[:], bq[:])
            nc.sync.dma_start(bk_t[:], bk[:])
            nc.sync.dma_start(bv_t[:], bv[:])
            wp_t = cpool.tile([128, D], F32R, tag="wp")
            nc.sync.dma_start(wp_t[:], wp[:])
            msk_t = cpool.tile([128, 4, 2 * TT], F32R, tag="msk")
            nc.sync.dma_start(msk_t[:], msk.rearrange("g p q -> p g q"))
            ones_t = cpool.tile([128, 64], F32R, tag="ones")
            nc.sync.dma_start(ones_t[:], ones[:])
            ident = cpool.tile([128, 128], F32, tag="ident")
            make_identity(nc, ident[:])

            def body():
                for b in range(B):
                    # ---------- QKV projections: qT/kT/vT in [e, t] ----------
                    qT = wpool.tile([128, S], F32R, tag="qT")
                    kT = wpool.tile([128, S], F32R, tag="kT")
                    vT = wpool.tile([128, S], F32R, tag="vT")
                    for tt_ in range(NQI):
                        ps_qk = ps2.tile([128, 1024], F32, tag="scores")
                        ps_v = ps2.tile([128, 1024], F32, tag="scores")
                        for dc in range(8):
                            xt_t = p3.tile([128, TT], F32R, tag="xtile")
                            nc.sync.dma_start(
                                xt_t[:], xt[b, ts(dc, 128), ts(tt_, TT)]
                            )
                            st = dc == 0
                            sp = dc == 7
                            nc.tensor.matmul(
                                ps_qk[:, 0:512], wq_t[:, dc], xt_t[:], start=st, stop=sp
                            )
                            nc.tensor.matmul(
                                ps_qk[:, 512:1024],
                                wk_t[:, dc],
                                xt_t[:],
                                start=st,
                                stop=sp,
                            )
                            nc.tensor.matmul(
                                ps_v[:, 0:512], wv_t[:, dc], xt_t[:], start=st, stop=sp
                            )
                        nc.scalar.activation(
                            qT[:, ts(tt_, TT)], ps_qk[:, 0:512], Identity, bias=bq_t[:]
                        )
                        nc.scalar.activation(
                            kT[:, ts(tt_, TT)],
                            ps_qk[:, 512:1024],
                            Identity,
                            bias=bk_t[:],
                        )
                        nc.scalar.activation(
                            vT[:, ts(tt_, TT)], ps_v[:, 0:512], Identity, bias=bv_t[:]
                        )

                    # ---------- v to natural layout [t, e] with ones column ----------
                    v_nat = wpool.tile([128, NKJ, 2, 65], F32R, tag="v_nat")
                    nc.vector.tensor_copy(
                        v_nat[:, :, :, 64:65],
                        ones_t[:, 0:32].rearrange("p (a h c) -> p a h c", a=NKJ, h=2),
                    )
                    for i in range(NKJ):
                        ps_t = ps2.tile([128, 512], F32, tag="misc")
                        nc.tensor.transpose(
                            ps_t[:, 0:128], vT[:, ts(i, 128)].bitcast(F32), ident[:]
                        )
                        nc.vector.tensor_copy(
                            v_nat[:, i, :, 0:64],
                            ps_t[:, 0:128].rearrange("p (h e) -> p h e", h=2),
                        )

                    # ---------- attention (both heads packed side by side) ----------
                    z2T = wpool.tile([128, S], F32R, tag="z2T")
                    for qi in range(NQI):
                        n_kj = 4 * qi + 4
                        zp0 = ps2.tile([65, 512], F32, tag="zprime")
                        zp1 = ps2.tile([65, 512], F32, tag="zprime")
                        zps = (zp0, zp1)
                        for kj in range(n_kj):
                            sc = ps2.tile([128, 1024], F32, tag="scores")
                            for h in (0, 1):
                                nc.tensor.matmul(
                                    sc[:, h * 512 : h * 512 + 512],
                                    kT[64 * h : 64 * h + 64, ts(kj, 128)],
                                    qT[64 * h : 64 * h + 64, ts(qi, TT)],
                                    start=True,
                                    stop=True,
                                )
                            pt = p3.tile([128, 1024], F32R, tag="pt")
                            nc.scalar.activation(pt[:], sc[:], Exp, scale=0.125)
                            g = kj - 4 * qi
                            if g >= 0:
                                nc.vector.tensor_mul(pt[:], pt[:], msk_t[:, g, :])
                            for h in (0, 1):
                                nc.tensor.matmul(
                                    zps[h][:],
                                    v_nat[:, kj, h, :],
                                    pt[:, h * 512 : h * 512 + 512],
                                    start=(kj == 0),
                                    stop=(kj == n_kj - 1),
                                )
                        for h in (0, 1):
                            rec = p2.tile([128, 512], F32R, tag="rec")
                            with nc.allow_low_precision(reason="fp32r softmax recip"):
                                nc.vector.reciprocal(
                                    rec[64:65, :], zps[h][64:65, :]
                                )
                            rb = ps2.tile([128, 512], F32, tag="misc")
                            nc.tensor.matmul(
                                rb[0:64, :],
                                ones_t[64:65, 0:64],
                                rec[64:65, :],
                                start=True,
                                stop=True,
                            )
                            rbs = p2.tile([64, 512], F32R, tag="rbs")
                            nc.vector.tensor_copy(rbs[:], rb[0:64, :])
                            if h == 0:
                                nc.vector.tensor_mul(
                                    z2T[0:64, ts(qi, TT)], zps[h][0:64, :], rbs[:]
                                )
                            else:
                                stg = p2.tile([64, 512], F32R, tag="stage")
                                nc.vector.tensor_mul(stg[:], zps[h][0:64, :], rbs[:])
                                nc.sync.dma_start(z2T[64:128, ts(qi, TT)], stg[:])

                    # ---------- partial output projection ----------
                    for i in range(NKJ):
                        pp = ps2.tile([128, 1024], F32, tag="scores")
                        for oh in (0, 1):
                            nc.tensor.matmul(
                                pp[:, oh * 512 : oh * 512 + 512],
                                z2T[:, ts(i, 128)],
                                wp_t[:, ts(oh, 512)],
                                start=True,
                                stop=True,
                            )
                        os_ = p3.tile([128, 1024], F32, tag="ostage")
                        nc.vector.tensor_copy(os_[:], pp[:])
                        nc.sync.dma_start(out[b, ts(i, 128), :], os_[:])

            if reps == 1:
                body()
            else:
                with tc.For_i(0, reps, 1):
                    body()

    _legalize_multi_waits(nc)
    return nc


def _host_inputs(x, W_qkv, b_qkv, W_proj):
    """Full inputs -> list of per-core input dicts."""
    x = np.asarray(x, dtype=np.float32)
    W_qkv = np.asarray(W_qkv, dtype=np.float32)
    b_qkv = np.asarray(b_qkv, dtype=np.float32)
    W_proj = np.asarray(W_proj, dtype=np.float32)

    xt = np.ascontiguousarray(x.transpose(0, 2, 1))  # [B, D, S]

    masks = np.empty((4, 128, 2 * TT), dtype=np.float32)
    qidx = np.arange(TT)[None, :]
    kidx = np.arange(128)[:, None]
    for g in range(4):
        m = (qidx >= kidx + 128 * g).astype(np.float32)
        masks[g] = np.concatenate([m, m], axis=1)
    ones = np.ones((128, 64), dtype=np.float32)

    in_maps = []
    for c in range(NCORES):
        cols = slice(128 * c, 128 * c + 128)
        in_maps.append(
            {
                "xt": xt,
                "wq": np.ascontiguousarray(W_qkv[:, cols]).reshape(8, 128, 128),
                "wk": np.ascontiguousarray(W_qkv[:, 1024:2048][:, cols]).reshape(
                    8, 128, 128
                ),
                "wv": np.ascontiguousarray(W_qkv[:, 2048:3072][:, cols]).reshape(
                    8, 128, 128
                ),
                "bq": np.ascontiguousarray(b_qkv[0:1024][cols]).reshape(128, 1),
                "bk": np.ascontiguousarray(b_qkv[1024:2048][cols]).reshape(128, 1),
                "bv": np.ascontiguousarray(b_qkv[2048:3072][cols]).reshape(128, 1),
                "wp": np.ascontiguousarray(W_proj[cols, :]),
                "msk": masks,
                "ones": ones,
            }
        )
    return in_maps


_module_cache = {}


def _get_module(reps: int = 1):
    if reps not in _module_cache:
        _module_cache[reps] = build_module(reps)
    return _module_cache[reps]


def run_on_device(in_maps, reps: int = 1):
    from concourse.bass_utils import run_bass_kernel_spmd

    nc = _get_module(reps)
    return run_bass_kernel_spmd(
        nc, in_maps, core_ids=list(range(NCORES)), trace=False
    )


def kernel(x, W_qkv, b_qkv, W_proj, b_proj):
    in_maps = _host_inputs(x, W_qkv, b_qkv, W_proj)
    res = run_on_device(in_maps, reps=1)
    out = np.zeros((B, S, D), dtype=np.float32)
    for c in range(NCORES):
        out += res.results[c]["out"]
    out += np.asarray(b_proj, dtype=np.float32)
    return out


# revision 18
# speedup vs baseline: 720.4341x; 720.4341x over previous
"""Masked multi-head attention on 8 Trainium2 NeuronCores.

Reference computation (fp32):
    qkv = x @ W_qkv + b_qkv ; split q,k,v ; 16 heads, dh=64
    attn = softmax(causal(q k^T / 8)) ; z = attn v ; out = z @ W_proj + b_proj

Sharding: tensor-parallel over heads. Core c owns heads {2c, 2c+1}
(columns 128c:128c+128 of each of the q/k/v blocks of W_qkv, rows
128c:128c+128 of W_proj). Each core computes its heads' attention and a
partial output projection; the host sums the 8 partials and adds b_proj.

On-device layout highlights:
  - x is fed pre-transposed (host) as xt[b] = x[b].T so the QKV matmuls
    contract over d with d on partitions: q^T = W_q^T @ x^T comes out in
    [e, t] layout directly (e = 2*64 head channels on partitions).
  - Scores are computed transposed, S^T[k, q] = (k^T)^T q^T per head, so
    softmax's denominator is produced by an ones-augmented AV matmul:
    lhsT = [v | 1] gives z'^T = [z^T ; sum_j P] in one accumulation.
  - exp (with the 1/8 scale folded in) runs on the scalar engine straight
    out of PSUM; causal masking multiplies precomputed 0/1 tiles on the
    diagonal blocks only; fully-masked tiles are never computed.
  - All matmuls run in fp32r (full PE rate at N>=512, ~2^-12 rounding).

The harness-visible entry point is kernel(**inputs) -> np.ndarray.
"""

import sys

sys.path.insert(0, "/opt/trn_rl_repo")

import numpy as np

B = 2
S = 2048
D = 1024
NH = 16
DH = 64
NCORES = 8
TT = 512  # t-tile for qkv / q-tile for scores
NQI = S // TT  # 4
NKJ = S // 128  # 16


def _legalize_multi_waits(nc, max_waits=1):
    """This container's walrus rejects >1 semaphore wait per instruction
    (CoreV3 setupSyncWait "Too many sync wait commands"). Hoist extras
    onto same-engine NOPs inserted right before the offending one."""
    import concourse.mybir as mybir

    n_fixed = 0
    for fn in nc.m.functions:
        for blk in fn.blocks:
            new_insts = []
            for inst in blk.instructions:
                si = inst.sync_info
                waits = list(si.on_wait) if si is not None else []
                if len(waits) > max_waits:
                    extra, keep = waits[:-max_waits], waits[-max_waits:]
                    k = 0
                    while extra:
                        chunk, extra = extra[:max_waits], extra[max_waits:]
                        new_insts.append(
                            mybir.InstNoOp(
                                name=f"{inst.name}-wsplit{k}",
                                engine=inst.engine,
                                ins=[],
                                outs=[],
                                sync_info=mybir.SyncInfo(on_wait=chunk, on_update=[]),
                            )
                        )
                        k += 1
                    inst.sync_info = mybir.SyncInfo(
                        on_wait=keep, on_update=list(si.on_update)
                    )
                    n_fixed += 1
                new_insts.append(inst)
            blk.instructions = new_insts
    return n_fixed


def build_module(reps: int = 1, cfg: dict | None = None):
    cfg = dict(cfg or {})
    qkv2 = cfg.get("qkv2", True)       # double-buffer qT/kT
    vnz2 = cfg.get("vnz2", True)       # double-buffer v_nat/z2T
    pt_bufs = cfg.get("pt_bufs", 2)
    ostage_bufs = cfg.get("ostage_bufs", 2)
    import concourse.bass as bass
    import concourse.mybir as mybir
    import concourse.tile as tile
    from concourse.bass import ts
    from concourse.masks import make_identity

    F32 = mybir.dt.float32
    F32R = mybir.dt.float32r
    BF16 = mybir.dt.bfloat16
    Identity = mybir.ActivationFunctionType.Identity
    Exp = mybir.ActivationFunctionType.Exp

    nc = bass.Bass(
        trn_type="TRN2", target_bir_lowering=False, debug=False, num_devices=NCORES
    )

    xt = nc.dram_tensor("xt", [B, D, S], F32R, kind="ExternalInput").ap()
    wq = nc.dram_tensor("wq", [8, 128, 128], F32R, kind="ExternalInput").ap()
    wk = nc.dram_tensor("wk", [8, 128, 128], F32R, kind="ExternalInput").ap()
    wv = nc.dram_tensor("wv", [8, 128, 128], F32R, kind="ExternalInput").ap()
    bq = nc.dram_tensor("bq", [128, 1], F32, kind="ExternalInput").ap()
    bk = nc.dram_tensor("bk", [128, 1], F32, kind="ExternalInput").ap()
    bv = nc.dram_tensor("bv", [128, 1], F32, kind="ExternalInput").ap()
    wp = nc.dram_tensor("wp", [128, D], F32R, kind="ExternalInput").ap()
    msk = nc.dram_tensor("msk", [4, 128, 2 * TT], F32R, kind="ExternalInput").ap()
    ones = nc.dram_tensor("ones", [128, 64], F32R, kind="ExternalInput").ap()
    out = nc.dram_tensor("out", [B, S, D], F32, kind="ExternalOutput").ap()

    with tile.TileContext(nc) as tc:
        with (
            tc.tile_pool(name="const", bufs=1) as cpool,
            tc.tile_pool(name="work", bufs=1) as wpool,
            tc.tile_pool(name="work2", bufs=2) as wpool2,
            tc.tile_pool(name="p3", bufs=3) as p3,
            tc.tile_pool(name="p2", bufs=2) as p2,
            tc.tile_pool(name="ppt", bufs=pt_bufs) as ppt,
            tc.tile_pool(name="pos", bufs=ostage_bufs) as pos,
            tc.tile_pool(name="ps2", bufs=2, space="PSUM") as ps2,
            tc.tile_pool(name="psz", bufs=2, space="PSUM") as psz,
        ):
            # ---- constants (outside the timing loop) ----
            wq_t = cpool.tile([128, 8, 128], F32R, tag="wq")
            wk_t = cpool.tile([128, 8, 128], F32R, tag="wk")
            wv_t = cpool.tile([128, 8, 128], F32R, tag="wv")
            nc.sync.dma_start(wq_t[:], wq.rearrange("o p e -> p o e"))
            nc.sync.dma_start(wk_t[:], wk.rearrange("o p e -> p o e"))
            nc.sync.dma_start(wv_t[:], wv.rearrange("o p e -> p o e"))
            bq_t = cpool.tile([128, 1], F32, tag="bq")
            bk_t = cpool.tile([128, 1], F32, tag="bk")
            bv_t = cpool.tile([128, 1], F32, tag="bv")
            nc.sync.dma_start(bq_t[:], bq[:])
            nc.sync.dma_start(bk_t[:], bk[:])
            nc.sync.dma_start(bv_t[:], bv[:])
            wp_t = cpool.tile([128, D], F32R, tag="wp")
            nc.sync.dma_start(wp_t[:], wp[:])
            msk_t = cpool.tile([128, 4, 2 * TT], F32R, tag="msk")
            nc.sync.dma_start(msk_t[:], msk.rearrange("g p q -> p g q"))
            ones_t = cpool.tile([128, 64], F32R, tag="ones")
            nc.sync.dma_start(ones_t[:], ones[:])
            ident = cpool.tile([128, 128], F32, tag="ident")
            make_identity(nc, ident[:])

            def body():
                for b in range(B):
                    # ---------- QKV projections: qT/kT/vT in [e, t] ----------
                    qT = (wpool2 if qkv2 else wpool).tile([128, S], F32R, tag="qT")
                    kT = (wpool2 if qkv2 else wpool).tile([128, S], F32R, tag="kT")
                    vT = wpool.tile([128, S], F32R, tag="vT")
                    xt_f = wpool.tile([128, 8, S], F32R, tag="xtf")
                    for dc in range(8):
                        nc.sync.dma_start(xt_f[:, dc, :], xt[b, ts(dc, 128), :])
                    for tt_ in range(NQI):
                        ps_qk = ps2.tile([128, 1024], F32, tag="scores")
                        ps_v = ps2.tile([128, 1024], F32, tag="scores")
                        for dc in range(8):
                            xt_t = xt_f[:, dc, ts(tt_, TT)]
                            st = dc == 0
                            sp = dc == 7
                            nc.tensor.matmul(
                                ps_qk[:, 0:512], wq_t[:, dc], xt_t, start=st, stop=sp
                            )
                            nc.tensor.matmul(
                                ps_qk[:, 512:1024],
                                wk_t[:, dc],
                                xt_t,
                                start=st,
                                stop=sp,
                            )
                            nc.tensor.matmul(
                                ps_v[:, 0:512], wv_t[:, dc], xt_t, start=st, stop=sp
                            )
                        nc.scalar.activation(
                            qT[:, ts(tt_, TT)], ps_qk[:, 0:512], Identity, bias=bq_t[:]
                        )
                        nc.scalar.activation(
                            kT[:, ts(tt_, TT)],
                            ps_qk[:, 512:1024],
                            Identity,
                            bias=bk_t[:],
                        )
                        nc.scalar.activation(
                            vT[:, ts(tt_, TT)], ps_v[:, 0:512], Identity, bias=bv_t[:]
                        )

                    # ---------- v to natural layout [t, e] with ones column ----------
                    v_nat = (wpool2 if vnz2 else wpool).tile([128, NKJ, 2, 65], F32R, tag="v_nat")
                    nc.vector.tensor_copy(
                        v_nat[:, :, :, 64:65],
                        ones_t[:, 0:32].rearrange("p (a h c) -> p a h c", a=NKJ, h=2),
                    )
                    for i in range(NKJ):
                        ps_t = ps2.tile([128, 512], F32, tag="misc")
                        nc.tensor.transpose(
                            ps_t[:, 0:128], vT[:, ts(i, 128)].bitcast(F32), ident[:]
                        )
                        nc.vector.tensor_copy(
                            v_nat[:, i, :, 0:64],
                            ps_t[:, 0:128].rearrange("p (h e) -> p h e", h=2),
                        )

                    # ---------- attention (both heads packed side by side) ----------
                    z2T = (wpool2 if vnz2 else wpool).tile([128, S], F32R, tag="z2T")
                    stg = wpool.tile([64, S], F32R, tag="stage")
                    for qi in range(NQI):
                        n_kj = 4 * qi + 4
                        zp0 = psz.tile([65, 512], F32, tag="zprime")
                        zp1 = psz.tile([65, 512], F32, tag="zprime")
                        zps = (zp0, zp1)
                        for kj in range(n_kj):
                            sc = ps2.tile([128, 1024], F32, tag="scores")
                            for h in (0, 1):
                                nc.tensor.matmul(
                                    sc[:, h * 512 : h * 512 + 512],
                                    kT[64 * h : 64 * h + 64, ts(kj, 128)],
                                    qT[64 * h : 64 * h + 64, ts(qi, TT)],
                                    start=True,
                                    stop=True,
                                )
                            pt = ppt.tile([128, 1024], F32R, tag="pt")
                            nc.scalar.activation(pt[:], sc[:], Exp, scale=0.125)
                            g = kj - 4 * qi
                            if g >= 0:
                                nc.vector.tensor_mul(pt[:], pt[:], msk_t[:, g, :])
                            for h in (0, 1):
                                nc.tensor.matmul(
                                    zps[h][:],
                                    v_nat[:, kj, h, :],
                                    pt[:, h * 512 : h * 512 + 512],
                                    start=(kj == 0),
                                    stop=(kj == n_kj - 1),
                                )
                        for h in (0, 1):
                            rec = p2.tile([128, 512], F32R, tag="rec")
                            with nc.allow_low_precision(reason="fp32r softmax recip"):
                                nc.vector.reciprocal(rec[64:65, :], zps[h][64:65, :])
                            rb = ps2.tile([128, 512], F32, tag="misc")
                            nc.tensor.matmul(
                                rb[0:64, :],
                                ones_t[64:65, 0:64],
                                rec[64:65, :],
                                start=True,
                                stop=True,
                            )
                            rbs = p2.tile([64, 512], F32R, tag="rbs")
                            nc.vector.tensor_copy(rbs[:], rb[0:64, :])
                            if h == 0:
                                nc.vector.tensor_mul(
                                    z2T[0:64, ts(qi, TT)], zps[h][0:64, :], rbs[:]
                                )
                            else:
                                nc.vector.tensor_mul(
                                    stg[:, ts(qi, TT)], zps[h][0:64, :], rbs[:]
                                )
                    nc.sync.dma_start(z2T[64:128, :], stg[:])

                    # ---------- partial output projection ----------
                    for i2 in range(NKJ // 2):
                        os_ = pos.tile([128, 2, D], F32, tag="ostage")
                        for j in (0, 1):
                            i = 2 * i2 + j
                            pp = ps2.tile([128, 1024], F32, tag="scores")
                            for oh in (0, 1):
                                nc.tensor.matmul(
                                    pp[:, oh * 512 : oh * 512 + 512],
                                    z2T[:, ts(i, 128)],
                                    wp_t[:, ts(oh, 512)],
                                    start=True,
                                    stop=True,
                                )
                            nc.vector.tensor_copy(os_[:, j, :], pp[:])
                        nc.sync.dma_start(
                            out[b, ts(i2, 256), :].rearrange("(j p) o -> p j o", p=128),
                            os_[:],
                        )

            if reps == 1:
                body()
            else:
                engs = (
                    mybir.EngineType.PE,
                    mybir.EngineType.Activation,
                    mybir.EngineType.DVE,
                    mybir.EngineType.SP,
                )
                with tc.For_i(0, reps, 1, hint_engines=engs):
                    body()

    _legalize_multi_waits(nc)
    return nc


def _host_inputs(x, W_qkv, b_qkv, W_proj):
    """Full inputs -> list of per-core input dicts."""
    x = np.asarray(x, dtype=np.float32)
    W_qkv = np.asarray(W_qkv, dtype=np.float32)
    b_qkv = np.asarray(b_qkv, dtype=np.float32)
    W_proj = np.asarray(W_proj, dtype=np.float32)

    xt = np.ascontiguousarray(x.transpose(0, 2, 1))  # [B, D, S]

    masks = np.empty((4, 128, 2 * TT), dtype=np.float32)
    qidx = np.arange(TT)[None, :]
    kidx = np.arange(128)[:, None]
    for g in range(4):
        m = (qidx >= kidx + 128 * g).astype(np.float32)
        masks[g] = np.concatenate([m, m], axis=1)

    ones = np.ones((128, 64), dtype=np.float32)

    in_maps = []
    for c in range(NCORES):
        cols = slice(128 * c, 128 * c + 128)
        in_maps.append(
            {
                "xt": xt,
                "wq": np.ascontiguousarray(W_qkv[:, 0:1024][:, cols]).reshape(
                    8, 128, 128
                ),
                "wk": np.ascontiguousarray(W_qkv[:, 1024:2048][:, cols]).reshape(
                    8, 128, 128
                ),
                "wv": np.ascontiguousarray(W_qkv[:, 2048:3072][:, cols]).reshape(
                    8, 128, 128
                ),
                "bq": np.ascontiguousarray(b_qkv[0:1024][cols]).reshape(128, 1),
                "bk": np.ascontiguousarray(b_qkv[1024:2048][cols]).reshape(128, 1),
                "bv": np.ascontiguousarray(b_qkv[2048:3072][cols]).reshape(128, 1),
                "wp": np.ascontiguousarray(W_proj[cols, :]),
                "msk": masks,
                "ones": ones,
            }
        )
    return in_maps


_module_cache = {}


BEST_CFG = {'qkv2': False, 'vnz2': False, 'pt_bufs': 4, 'ostage_bufs': 4}


def _get_module(reps: int = 1):
    if reps not in _module_cache:
        _module_cache[reps] = build_module(reps, BEST_CFG)
    return _module_cache[reps]


def run_on_device(in_maps, reps: int = 1):
    from concourse.bass_utils import run_bass_kernel_spmd

    nc = _get_module(reps)
    return run_bass_kernel_spmd(nc, in_maps, core_ids=list(range(NCORES)), trace=False)


def kernel(x, W_qkv, b_qkv, W_proj, b_proj):
    in_maps = _host_inputs(x, W_qkv, b_qkv, W_proj)
    res = run_on_device(in_maps, reps=1)
    out = np.zeros((B, S, D), dtype=np.float32)
    for c in range(NCORES):
        out += res.results[c]["out"]
    out += np.asarray(b_proj, dtype=np.float32)
    return out


# revision 26
# speedup vs baseline: 1661.0199x; 2.3056x over previous
"""Masked multi-head attention on 8 Trainium2 NeuronCores.

Reference computation (fp32):
    qkv = x @ W_qkv + b_qkv ; split q,k,v ; 16 heads, dh=64
    attn = softmax(causal(q k^T / 8)) ; z = attn v ; out = z @ W_proj + b_proj

Sharding: tensor-parallel over heads. Core c owns heads {2c, 2c+1}
(columns 128c:128c+128 of each of the q/k/v blocks of W_qkv, rows
128c:128c+128 of W_proj). Each core computes its heads' attention and a
partial output projection; the host sums the 8 partials and adds b_proj.

On-device layout highlights:
  - x is fed pre-transposed (host) as xt[b] = x[b].T so the QKV matmuls
    contract over d with d on partitions: q^T = W_q^T @ x^T comes out in
    [e, t] layout directly (e = 2*64 head channels on partitions).
  - Scores are computed transposed, S^T[k, q] = (k^T)^T q^T per head, so
    softmax's denominator is produced by an ones-augmented AV matmul:
    lhsT = [v | 1] gives z'^T = [z^T ; sum_j P] in one accumulation.
  - exp (with the 1/8 scale folded in) runs on the scalar engine straight
    out of PSUM; causal masking multiplies precomputed 0/1 tiles on the
    diagonal blocks only; fully-masked tiles are never computed.
  - All matmuls run in fp32r (full PE rate at N>=512, ~2^-12 rounding).

The harness-visible entry point is kernel(**inputs) -> np.ndarray.
"""

import sys

sys.path.insert(0, "/opt/trn_rl_repo")

import numpy as np

B = 2
S = 2048
D = 1024
NH = 16
DH = 64
NCORES = 8
TT = 512  # t-tile for qkv / q-tile for scores
NQI = S // TT  # 4
NKJ = S // 128  # 16


def _legalize_multi_waits(nc, max_waits=1):
    """This container's walrus rejects >1 semaphore wait per instruction
    (CoreV3 setupSyncWait "Too many sync wait commands"). Hoist extras
    onto same-engine NOPs inserted right before the offending one."""
    import concourse.mybir as mybir

    n_fixed = 0
    for fn in nc.m.functions:
        for blk in fn.blocks:
            new_insts = []
            for inst in blk.instructions:
                si = inst.sync_info
                waits = list(si.on_wait) if si is not None else []
                if len(waits) > max_waits:
                    extra, keep = waits[:-max_waits], waits[-max_waits:]
                    k = 0
                    while extra:
                        chunk, extra = extra[:max_waits], extra[max_waits:]
                        new_insts.append(
                            mybir.InstNoOp(
                                name=f"{inst.name}-wsplit{k}",
                                engine=inst.engine,
                                ins=[],
                                outs=[],
                                sync_info=mybir.SyncInfo(on_wait=chunk, on_update=[]),
                            )
                        )
                        k += 1
                    inst.sync_info = mybir.SyncInfo(
                        on_wait=keep, on_update=list(si.on_update)
                    )
                    n_fixed += 1
                new_insts.append(inst)
            blk.instructions = new_insts
    return n_fixed


def build_module(reps: int = 1, cfg: dict | None = None):
    cfg = dict(cfg or {})
    qkv2 = cfg.get("qkv2", True)       # double-buffer qT/kT
    vnz2 = cfg.get("vnz2", True)       # double-buffer v_nat/z2T
    pt_bufs = cfg.get("pt_bufs", 2)
    ostage_bufs = cfg.get("ostage_bufs", 2)
    import concourse.bass as bass
    import concourse.mybir as mybir
    import concourse.tile as tile
    from concourse.bass import ts
    from concourse.masks import make_identity

    F32 = mybir.dt.float32
    F32R = mybir.dt.float32r
    BF16 = mybir.dt.bfloat16
    Identity = mybir.ActivationFunctionType.Identity
    Exp = mybir.ActivationFunctionType.Exp

    nc = bass.Bass(
        trn_type="TRN2", target_bir_lowering=False, debug=False, num_devices=NCORES
    )

    xt = nc.dram_tensor("xt", [B, D, S], F32R, kind="ExternalInput").ap()
    wq = nc.dram_tensor("wq", [8, 128, 128], F32R, kind="ExternalInput").ap()
    wk = nc.dram_tensor("wk", [8, 128, 128], F32R, kind="ExternalInput").ap()
    wv = nc.dram_tensor("wv", [8, 128, 128], F32R, kind="ExternalInput").ap()
    bq = nc.dram_tensor("bq", [128, 1], F32, kind="ExternalInput").ap()
    bk = nc.dram_tensor("bk", [128, 1], F32, kind="ExternalInput").ap()
    bv = nc.dram_tensor("bv", [128, 1], F32, kind="ExternalInput").ap()
    wp = nc.dram_tensor("wp", [128, D], F32R, kind="ExternalInput").ap()
    msk = nc.dram_tensor("msk", [4, 128, 2 * TT], F32R, kind="ExternalInput").ap()
    ones = nc.dram_tensor("ones", [128, 64], F32R, kind="ExternalInput").ap()
    out = nc.dram_tensor("out", [B, S, D], F32, kind="ExternalOutput").ap()

    with tile.TileContext(nc) as tc:
        with (
            tc.tile_pool(name="const", bufs=1) as cpool,
            tc.tile_pool(name="work", bufs=1) as wpool,
            tc.tile_pool(name="work2", bufs=2) as wpool2,
            tc.tile_pool(name="p3", bufs=3) as p3,
            tc.tile_pool(name="p2", bufs=2) as p2,
            tc.tile_pool(name="ppt", bufs=pt_bufs) as ppt,
            tc.tile_pool(name="pos", bufs=ostage_bufs) as pos,
            tc.tile_pool(name="ps2", bufs=2, space="PSUM") as ps2,
            tc.tile_pool(name="psm", bufs=1, space="PSUM") as psm,
            tc.tile_pool(name="psz", bufs=3, space="PSUM") as psz,
        ):
            # ---- constants (outside the timing loop) ----
            wq_t = cpool.tile([128, 8, 128], F32R, tag="wq")
            wk_t = cpool.tile([128, 8, 128], F32R, tag="wk")
            wv_t = cpool.tile([128, 8, 128], F32R, tag="wv")
            nc.sync.dma_start(wq_t[:], wq.rearrange("o p e -> p o e"))
            nc.sync.dma_start(wk_t[:], wk.rearrange("o p e -> p o e"))
            nc.sync.dma_start(wv_t[:], wv.rearrange("o p e -> p o e"))
            bq_t = cpool.tile([128, 1], F32, tag="bq")
            bk_t = cpool.tile([128, 1], F32, tag="bk")
            bv_t = cpool.tile([128, 1], F32, tag="bv")
            nc.sync.dma_start(bq_t[:], bq[:])
            nc.sync.dma_start(bk_t[:], bk[:])
            nc.sync.dma_start(bv_t[:], bv[:])
            wp_t = cpool.tile([128, D], F32R, tag="wp")
            nc.sync.dma_start(wp_t[:], wp[:])
            msk_t = cpool.tile([128, 4, 2 * TT], F32R, tag="msk")
            nc.sync.dma_start(msk_t[:], msk.rearrange("g p q -> p g q"))
            ones_t = cpool.tile([128, 64], F32R, tag="ones")
            nc.sync.dma_start(ones_t[:], ones[:])
            ident = cpool.tile([128, 128], F32, tag="ident")
            make_identity(nc, ident[:])

            def body():
                for b in range(B):
                    # ---------- QKV projections: qT/kT/vT in [e, t] ----------
                    qT = (wpool2 if qkv2 else wpool).tile([128, S], F32R, tag="qT")
                    kT = (wpool2 if qkv2 else wpool).tile([128, S], F32R, tag="kT")
                    vT = wpool.tile([128, S], F32R, tag="vT")
                    xt_f = wpool.tile([128, 8, S], F32R, tag="xtf")
                    dma_eng = nc.gpsimd if reps == 1 else nc.sync
                    for dc in range(8):
                        dma_eng.dma_start(xt_f[:, dc, :], xt[b, ts(dc, 128), :])
                    for tt_ in range(NQI):
                        ps_qk = ps2.tile([128, 1024], F32, tag="scores")
                        for dc in range(8):
                            xt_t = xt_f[:, dc, ts(tt_, TT)]
                            st = dc == 0
                            sp = dc == 7
                            nc.tensor.matmul(
                                ps_qk[:, 0:512], wq_t[:, dc], xt_t, start=st, stop=sp
                            )
                            nc.tensor.matmul(
                                ps_qk[:, 512:1024],
                                wk_t[:, dc],
                                xt_t,
                                start=st,
                                stop=sp,
                            )
                        nc.scalar.activation(
                            qT[:, ts(tt_, TT)], ps_qk[:, 0:512], Identity, bias=bq_t[:]
                        )
                        nc.scalar.activation(
                            kT[:, ts(tt_, TT)],
                            ps_qk[:, 512:1024],
                            Identity,
                            bias=bk_t[:],
                        )
                    for tp in range(NQI // 2):
                        ps_v = ps2.tile([128, 1024], F32, tag="scores")
                        for dc in range(8):
                            st = dc == 0
                            sp = dc == 7
                            for j in (0, 1):
                                nc.tensor.matmul(
                                    ps_v[:, j * 512 : j * 512 + 512],
                                    wv_t[:, dc],
                                    xt_f[:, dc, ts(2 * tp + j, TT)],
                                    start=st,
                                    stop=sp,
                                )
                        for j in (0, 1):
                            nc.scalar.activation(
                                vT[:, ts(2 * tp + j, TT)],
                                ps_v[:, j * 512 : j * 512 + 512],
                                Identity,
                                bias=bv_t[:],
                            )

                    # ---------- v to natural layout [t, e] with ones column ----------
                    v_nat = (wpool2 if vnz2 else wpool).tile([128, NKJ, 2, 65], F32R, tag="v_nat")
                    nc.vector.tensor_copy(
                        v_nat[:, :, :, 64:65],
                        ones_t[:, 0:32].rearrange("p (a h c) -> p a h c", a=NKJ, h=2),
                    )
                    for i in range(NKJ):
                        ps_t = psm.tile([128, 512], F32, tag="misc")
                        nc.tensor.transpose(
                            ps_t[:, 0:128], vT[:, ts(i, 128)].bitcast(F32), ident[:]
                        )
                        nc.vector.tensor_copy(
                            v_nat[:, i, :, 0:64],
                            ps_t[:, 0:128].rearrange("p (h e) -> p h e", h=2),
                        )

                    # ---------- attention (both heads packed side by side) ----------
                    z2T = (wpool2 if vnz2 else wpool).tile([128, S], F32R, tag="z2T")
                    stg = wpool.tile([64, S], F32R, tag="stage")
                    for qi in range(NQI):
                        n_kj = 4 * qi + 4
                        zp0 = psz.tile([65, 512], F32, tag="zprime")
                        zp1 = psz.tile([65, 512], F32, tag="zprime")
                        zps = (zp0, zp1)
                        for kj in range(n_kj):
                            sc = ps2.tile([128, 1024], F32, tag="scores")
                            for h in (0, 1):
                                nc.tensor.matmul(
                                    sc[:, h * 512 : h * 512 + 512],
                                    kT[64 * h : 64 * h + 64, ts(kj, 128)],
                                    qT[64 * h : 64 * h + 64, ts(qi, TT)],
                                    start=True,
                                    stop=True,
                                )
                            pt = ppt.tile([128, 1024], F32R, tag="pt")
                            nc.scalar.activation(pt[:], sc[:], Exp, scale=0.125)
                            g = kj - 4 * qi
                            if g >= 0:
                                nc.vector.tensor_mul(pt[:], pt[:], msk_t[:, g, :])
                            for h in (0, 1):
                                nc.tensor.matmul(
                                    zps[h][:],
                                    v_nat[:, kj, h, :],
                                    pt[:, h * 512 : h * 512 + 512],
                                    start=(kj == 0),
                                    stop=(kj == n_kj - 1),
                                )
                        for h in (0, 1):
                            rec = p2.tile([128, 512], F32R, tag="rec")
                            with nc.allow_low_precision(reason="fp32r softmax recip"):
                                nc.vector.reciprocal(rec[64:65, :], zps[h][64:65, :])
                            rb = psm.tile([128, 512], F32, tag="misc")
                            nc.tensor.matmul(
                                rb[0:64, :],
                                ones_t[64:65, 0:64],
                                rec[64:65, :],
                                start=True,
                                stop=True,
                            )
                            rbs = p2.tile([64, 512], F32R, tag="rbs")
                            nc.vector.tensor_copy(rbs[:], rb[0:64, :])
                            if h == 0:
                                nc.vector.tensor_mul(
                                    z2T[0:64, ts(qi, TT)], zps[h][0:64, :], rbs[:]
                                )
                            else:
                                nc.vector.tensor_mul(
                                    stg[:, ts(qi, TT)], zps[h][0:64, :], rbs[:]
                                )
                                nc.sync.dma_start(
                                    z2T[64:128, ts(qi, TT)], stg[:, ts(qi, TT)]
                                )

                    # ---------- partial output projection ----------
                    for i2 in range(NKJ // 2):
                        os_ = pos.tile([128, 2, D], F32, tag="ostage")
                        for j in (0, 1):
                            i = 2 * i2 + j
                            for oh in (0, 1):
                                pp = psz.tile([128, 512], F32, tag="zprime")
                                nc.tensor.matmul(
                                    pp[:],
                                    z2T[:, ts(i, 128)],
                                    wp_t[:, ts(oh, 512)],
                                    start=True,
                                    stop=True,
                                )
                                nc.vector.tensor_copy(
                                    os_[:, j, oh * 512 : oh * 512 + 512], pp[:]
                                )
                        nc.sync.dma_start(
                            out[b, ts(i2, 256), :].rearrange("(j p) o -> p j o", p=128),
                            os_[:],
                        )

            if reps == 1:
                body()
            else:
                engs = (
                    mybir.EngineType.PE,
                    mybir.EngineType.Activation,
                    mybir.EngineType.DVE,
                    mybir.EngineType.SP,
                )
                with tc.For_i(0, reps, 1, hint_engines=engs):
                    body()

    _legalize_multi_waits(nc)
    return nc


def _host_inputs(x, W_qkv, b_qkv, W_proj):
    """Full inputs -> list of per-core input dicts."""
    x = np.asarray(x, dtype=np.float32)
    W_qkv = np.asarray(W_qkv, dtype=np.float32)
    b_qkv = np.asarray(b_qkv, dtype=np.float32)
    W_proj = np.asarray(W_proj, dtype=np.float32)

    xt = np.ascontiguousarray(x.transpose(0, 2, 1))  # [B, D, S]

    masks = np.empty((4, 128, 2 * TT), dtype=np.float32)
    qidx = np.arange(TT)[None, :]
    kidx = np.arange(128)[:, None]
    for g in range(4):
        m = (qidx >= kidx + 128 * g).astype(np.float32)
        masks[g] = np.concatenate([m, m], axis=1)

    ones = np.ones((128, 64), dtype=np.float32)

    in_maps = []
    for c in range(NCORES):
        cols = slice(128 * c, 128 * c + 128)
        in_maps.append(
            {
                "xt": xt,
                "wq": np.ascontiguousarray(W_qkv[:, 0:1024][:, cols]).reshape(
                    8, 128, 128
                ),
                "wk": np.ascontiguousarray(W_qkv[:, 1024:2048][:, cols]).reshape(
                    8, 128, 128
                ),
                "wv": np.ascontiguousarray(W_qkv[:, 2048:3072][:, cols]).reshape(
                    8, 128, 128
                ),
                "bq": np.ascontiguousarray(b_qkv[0:1024][cols]).reshape(128, 1),
                "bk": np.ascontiguousarray(b_qkv[1024:2048][cols]).reshape(128, 1),
                "bv": np.ascontiguousarray(b_qkv[2048:3072][cols]).reshape(128, 1),
                "wp": np.ascontiguousarray(W_proj[cols, :]),
                "msk": masks,
                "ones": ones,
            }
        )
    return in_maps


_module_cache = {}


BEST_CFG = {'qkv2': False, 'vnz2': False, 'pt_bufs': 4, 'ostage_bufs': 4}


def _get_module(reps: int = 1):
    if reps not in _module_cache:
        _module_cache[reps] = build_module(reps, BEST_CFG)
    return _module_cache[reps]


def run_on_device(in_maps, reps: int = 1):
    from concourse.bass_utils import run_bass_kernel_spmd

    nc = _get_module(reps)
    return run_bass_kernel_spmd(nc, in_maps, core_ids=list(range(NCORES)), trace=False)


def kernel(x, W_qkv, b_qkv, W_proj, b_proj):
    in_maps = _host_inputs(x, W_qkv, b_qkv, W_proj)
    res = run_on_device(in_maps, reps=1)
    out = np.zeros((B, S, D), dtype=np.float32)
    for c in range(NCORES):
        out += res.results[c]["out"]
    out += np.asarray(b_proj, dtype=np.float32)
    return out
